# revision 5
# baseline (speedup 1.0000x reference)
"""Block-diagonal ZF equalizer (nn_BDEqualizer) as a Trainium2 Bass kernel.

Math: for every resource element (b, s, f) and UE u, solve the 8x8 complex
system H_u x_u = y_u where H_u[i, j] = h[b, 0, 8u+i, u, j, s, f] and
y_u[i] = y[b, 0, 8u+i, s, f].  Output x as [B, 1, 32, S, F, 2] (re/im last).

Strategy (data-parallel over the fft axis, per the sharding hint):
  - 8 cores, each owns a contiguous 128-subcarrier slice of F=1024.
  - Host pre-extracts the block-diagonal channel blocks (pure indexing) and
    ships per-core shards hd[B, U, 8, 8, S, 128] / yd[B, U, 8, S, 128].
  - On-chip layout: subcarriers on the 128 SBUF partitions, the other RE
    axes (u, b-pair, s) = 112 along the free dim.  Each of the 9 augmented
    matrix columns (8 of H + rhs) is a "plane" of 8 rows; every Gaussian
    elimination step is a full-width [128, n*112] elementwise op on the
    Vector engine, with per-RE pivot reciprocals.  Unpivoted LU + Jordan
    back-substitution, complex arithmetic as separate re/im tiles.
  - TensorE transposes move between the DMA-friendly [(u,b,s), f] staging
    layout and the compute layout [f, (u,b,s)]; ScalarE drains PSUM.
  - Two chunks (b in {0,1} then {2,3}) double-buffer load against compute.
"""

import os

import numpy as np

import concourse.bacc as bacc
import concourse.bass as bass
import concourse.mybir as mybir
from concourse.bass_utils import run_bass_kernel_spmd
from concourse.masks import make_identity
from concourse.tile import TileContext

B, NRX, NR, U, A, S, F = 4, 1, 32, 4, 8, 14, 1024
NCORES = 8
FS = F // NCORES        # 128 subcarriers per core
NB = 2                  # batch entries per chunk
NCH = B // NB           # chunks per core
M = U * NB * S          # 112 RE columns per chunk (u, b, s)
NP = 9                  # augmented planes: 8 matrix columns + rhs
F32 = mybir.dt.float32
AL = mybir.AluOpType

LAST_RESULTS = None     # BassKernelResults of the most recent run (for test.py)


def _off(j, i):
    """Free-dim offset of (plane j, row i) inside an H supertile."""
    return (j * A + i) * M


def _build():
    nc = bacc.Bacc(trn_type="TRN2")

    # Host-prepped layouts, chosen so every per-(chunk, i) DMA slice is
    # stride-collapsible: hd[i, u, b, s, j, f], yd[i, u, b, s, f],
    # out[i, u, b, s, f, c].  (i = matrix row, j = matrix column.)
    hdre = nc.dram_tensor("hd_re", [A, U, B, S, A, FS], F32, kind="ExternalInput")
    hdim = nc.dram_tensor("hd_im", [A, U, B, S, A, FS], F32, kind="ExternalInput")
    ydre = nc.dram_tensor("yd_re", [A, U, B, S, FS], F32, kind="ExternalInput")
    ydim = nc.dram_tensor("yd_im", [A, U, B, S, FS], F32, kind="ExternalInput")
    out = nc.dram_tensor("out", [A, U, B, S, FS, 2], F32, kind="ExternalOutput")

    with TileContext(nc) as tc:
        with (
            tc.tile_pool(name="consts", bufs=1) as consts,
            tc.tile_pool(name="supers", bufs=2) as supers,
            tc.tile_pool(name="work", bufs=1) as work,
            tc.tile_pool(name="stg", bufs=3) as stg,
            tc.tile_pool(name="stgo", bufs=3) as stgo,
            tc.tile_pool(name="psin", bufs=3, space="PSUM") as psin,
            tc.tile_pool(name="psy", bufs=2, space="PSUM") as psy_pool,
            tc.tile_pool(name="pso", bufs=2, space="PSUM") as pso_pool,
        ):
            ident = consts.tile([128, 128], F32)
            make_identity(nc, ident)

            for ci in range(NCH):
                b0 = ci * NB
                HRe = supers.tile([128, NP * A * M], F32, tag="HRe")
                HIm = supers.tile([128, NP * A * M], F32, tag="HIm")
                hsup = (HRe, HIm)

                def row(T, j, i):
                    return T[:, _off(j, i) : _off(j, i) + M]

                def rows3(T, j, i0, n):
                    base = _off(j, i0)
                    return T[:, base : base + n * M].rearrange(
                        "p (r c) -> p r c", r=n
                    )

                def bc(ap, n):
                    return ap[:, None, :].broadcast_to([128, n, M])

                # ---------------- load h ----------------
                for comp in range(2):
                    hsrc = (hdre, hdim)[comp]
                    for i in range(A):
                        stage = stg.tile([M, A * FS], F32, tag="stage")
                        src = hsrc[i, :, b0 : b0 + NB]
                        nc.sync.dma_start(stage, src)
                        for jg in range(2):
                            ps = psin.tile([128, 4 * M], F32, tag="psin")
                            for q in range(4):
                                j = jg * 4 + q
                                nc.tensor.transpose(
                                    ps[:, q * M : (q + 1) * M],
                                    stage[:, j * FS : (j + 1) * FS],
                                    ident[:M, :M],
                                )
                            base = _off(jg * 4, i)
                            dst = hsup[comp][:, base : base + 4 * A * M].rearrange(
                                "p (q c) -> p q c", q=4
                            )[:, :, :M]
                            src3 = ps.rearrange("p (q c) -> p q c", q=4)
                            nc.scalar.copy(dst, src3)

                # ---------------- load y ----------------
                for comp in range(2):
                    ysrc = (ydre, ydim)[comp]
                    for i in range(A):
                        sy = stg.tile([M, FS], F32, tag="stagey")
                        nc.sync.dma_start(sy, ysrc[i, :, b0 : b0 + NB])
                        py = psy_pool.tile([128, M], F32, tag="psy")
                        nc.tensor.transpose(py, sy, ident[:M, :M])
                        nc.scalar.copy(row(hsup[comp], 8, i), py)

                # ---------------- solve ----------------
                IR = work.tile([128, A * M], F32, tag="IR")
                II = work.tile([128, A * M], F32, tag="II")
                FRe = work.tile([128, (A - 1) * M], F32, tag="FRe")
                FIm = work.tile([128, (A - 1) * M], F32, tag="FIm")
                PAs = work.tile([128, (A - 1) * M], F32, tag="PAs")
                PBs = work.tile([128, (A - 1) * M], F32, tag="PBs")
                PCs = work.tile([128, (A - 1) * M], F32, tag="PCs")
                PDs = work.tile([128, (A - 1) * M], F32, tag="PDs")
                TD = work.tile([128, M], F32, tag="TD")
                TU = work.tile([128, M], F32, tag="TU")
                TR = work.tile([128, M], F32, tag="TR")

                def sc3(T, n):
                    return T[:, : n * M].rearrange("p (r c) -> p r c", r=n)

                # forward elimination
                for k in range(A):
                    a = row(HRe, k, k)
                    b_ = row(HIm, k, k)
                    nc.vector.tensor_mul(TD, a, a)
                    nc.vector.tensor_mul(TU, b_, b_)
                    nc.vector.tensor_add(TD, TD, TU)
                    nc.vector.reciprocal(TR, TD)
                    irk = IR[:, k * M : (k + 1) * M]
                    iik = II[:, k * M : (k + 1) * M]
                    nc.vector.tensor_mul(irk, a, TR)
                    nc.vector.tensor_mul(iik, b_, TR)
                    n = A - 1 - k
                    if n == 0:
                        continue
                    car = rows3(HRe, k, k + 1, n)
                    cai = rows3(HIm, k, k + 1, n)
                    irB = bc(irk, n)
                    iiB = bc(iik, n)
                    pa, pb, pc, pd = (sc3(t, n) for t in (PAs, PBs, PCs, PDs))
                    nc.vector.tensor_mul(pa, car, irB)  # a*ir
                    nc.vector.tensor_mul(pb, cai, iiB)  # b*ii
                    nc.vector.tensor_mul(pc, car, iiB)  # a*ii
                    nc.vector.tensor_mul(pd, cai, irB)  # b*ir
                    fre = sc3(FRe, n)
                    fim = sc3(FIm, n)
                    # F = -H[i,k] * inv(p):  fre = -(pa+pb), fim = pc-pd
                    nc.vector.scalar_tensor_tensor(
                        fre, pa, -1.0, pb, AL.mult, AL.subtract
                    )
                    nc.vector.tensor_sub(fim, pc, pd)
                    for j in range(k + 1, NP):
                        hr = rows3(HRe, j, k + 1, n)
                        hi = rows3(HIm, j, k + 1, n)
                        Br = bc(row(HRe, j, k), n)
                        Bi = bc(row(HIm, j, k), n)
                        nc.vector.tensor_mul(pa, fre, Br)
                        nc.vector.tensor_mul(pb, fim, Bi)
                        nc.vector.tensor_mul(pc, fre, Bi)
                        nc.vector.tensor_mul(pd, fim, Br)
                        # H[i,j] += F*B (complex)
                        nc.vector.tensor_add(hr, hr, pa)
                        nc.vector.tensor_sub(hr, hr, pb)
                        nc.vector.tensor_add(hi, hi, pc)
                        nc.vector.tensor_add(hi, hi, pd)

                # back substitution (Jordan): x_k = y_k*invp, then clear col k
                for k in range(A - 1, -1, -1):
                    yr = row(HRe, 8, k)
                    yi = row(HIm, 8, k)
                    irk = IR[:, k * M : (k + 1) * M]
                    iik = II[:, k * M : (k + 1) * M]
                    t1 = PAs[:, :M]
                    t2 = PBs[:, :M]
                    t3 = PCs[:, :M]
                    t4 = PDs[:, :M]
                    nc.vector.tensor_mul(t1, yr, irk)
                    nc.vector.tensor_mul(t2, yi, iik)
                    nc.vector.tensor_mul(t3, yi, irk)
                    nc.vector.tensor_mul(t4, yr, iik)
                    # x = y * conj(p)/|p|^2
                    nc.vector.tensor_add(yr, t1, t2)
                    nc.vector.tensor_sub(yi, t3, t4)
                    if k == 0:
                        continue
                    cr = rows3(HRe, k, 0, k)
                    ci_ = rows3(HIm, k, 0, k)
                    xrB = bc(yr, k)
                    xiB = bc(yi, k)
                    qa, qb, qc, qd = (sc3(t, k) for t in (PAs, PBs, PCs, PDs))
                    nc.vector.tensor_mul(qa, cr, xrB)
                    nc.vector.tensor_mul(qb, ci_, xiB)
                    nc.vector.tensor_mul(qc, cr, xiB)
                    nc.vector.tensor_mul(qd, ci_, xrB)
                    ytr = rows3(HRe, 8, 0, k)
                    yti = rows3(HIm, 8, 0, k)
                    # y_i -= H[i,k] * x_k
                    nc.vector.tensor_sub(ytr, ytr, qa)
                    nc.vector.tensor_add(ytr, ytr, qb)
                    nc.vector.tensor_sub(yti, yti, qc)
                    nc.vector.tensor_sub(yti, yti, qd)

                # ---------------- store ----------------
                for i in range(A):
                    so = stgo.tile([M, 2 * FS], F32, tag="so")
                    so3 = so.rearrange("p (f c) -> p f c", c=2)
                    for comp in range(2):
                        po = pso_pool.tile([M, FS], F32, tag="pso")
                        nc.tensor.transpose(
                            po, row(hsup[comp], 8, i), ident[:128, :128]
                        )
                        nc.scalar.copy(so3[:, :, comp], po)
                    dst = out[i, :, b0 : b0 + NB]
                    nc.sync.dma_start(dst, so)

    nc.finalize()
    return nc


_NC_CACHE = None


def _get_nc():
    global _NC_CACHE
    if _NC_CACHE is None:
        _NC_CACHE = _build()
    return _NC_CACHE


def _prep_core(y_re, y_im, h_re, h_im, c):
    """Host-side shard prep for core c: f-slice + block-diagonal extraction."""
    fsl = slice(c * FS, (c + 1) * FS)
    ue = np.arange(U)
    maps = {}
    for name, h in (("hd_re", h_re), ("hd_im", h_im)):
        h6 = h[:, 0, :, :, :, :, fsl].reshape(B, U, A, U, A, S, FS)
        hd = h6[:, ue, :, ue]              # [u, b, i, j, s, f]
        maps[name] = np.ascontiguousarray(
            hd.transpose(2, 0, 1, 4, 3, 5), dtype=np.float32
        )                                   # [i, u, b, s, j, f]
    for name, y in (("yd_re", y_re), ("yd_im", y_im)):
        y5 = y[:, 0, :, :, fsl].reshape(B, U, A, S, FS)   # [b, u, i, s, f]
        maps[name] = np.ascontiguousarray(
            y5.transpose(2, 1, 0, 3, 4), dtype=np.float32
        )                                   # [i, u, b, s, f]
    return maps


def kernel(y_re, y_im, h_re, h_im, **_ignored):
    global LAST_RESULTS
    y_re = np.asarray(y_re, dtype=np.float32)
    y_im = np.asarray(y_im, dtype=np.float32)
    h_re = np.asarray(h_re, dtype=np.float32)
    h_im = np.asarray(h_im, dtype=np.float32)

    nc = _get_nc()
    in_maps = [_prep_core(y_re, y_im, h_re, h_im, c) for c in range(NCORES)]
    trace = bool(int(os.environ.get("BD_TRACE", "0")))
    res = run_bass_kernel_spmd(
        nc, in_maps, core_ids=list(range(NCORES)), trace=trace
    )
    LAST_RESULTS = res
    outs = []
    for r in res.results:
        o = r["out"]                              # [i, u, b, s, f, c]
        o = o.transpose(2, 1, 0, 3, 4, 5)         # [b, u, i, s, f, c]
        outs.append(o.reshape(B, NR, S, FS, 2))
    full = np.concatenate(outs, axis=3)           # [B, NR, S, F, 2]
    return np.ascontiguousarray(full[:, None])    # [B, 1, NR, S, F, 2]


# revision 8
# speedup vs baseline: 1.0422x; 1.0422x over previous
"""Block-diagonal ZF equalizer (nn_BDEqualizer) as a Trainium2 Bass kernel.

Math: for every resource element (b, s, f) and UE u, solve the 8x8 complex
system H_u x_u = y_u where H_u[i, j] = h[b, 0, 8u+i, u, j, s, f] and
y_u[i] = y[b, 0, 8u+i, s, f].  Output x as [B, 1, 32, S, F, 2] (re/im last).

Strategy (data-parallel over the fft axis, per the sharding hint):
  - 8 cores, each owns a contiguous 128-subcarrier slice of F=1024.
  - Host pre-extracts the block-diagonal channel blocks (pure indexing) and
    ships per-core shards hd[B, U, 8, 8, S, 128] / yd[B, U, 8, S, 128].
  - On-chip layout: subcarriers on the 128 SBUF partitions, the other RE
    axes (u, b-pair, s) = 112 along the free dim.  Each of the 9 augmented
    matrix columns (8 of H + rhs) is a "plane" of 8 rows; every Gaussian
    elimination step is a full-width [128, n*112] elementwise op on the
    Vector engine, with per-RE pivot reciprocals.  Unpivoted LU + Jordan
    back-substitution, complex arithmetic as separate re/im tiles.
  - TensorE transposes move between the DMA-friendly [(u,b,s), f] staging
    layout and the compute layout [f, (u,b,s)]; ScalarE drains PSUM.
  - Two chunks (b in {0,1} then {2,3}) double-buffer load against compute.
"""

import os

import numpy as np

import concourse.bacc as bacc
import concourse.bass as bass
import concourse.mybir as mybir
from concourse.bass_utils import run_bass_kernel_spmd
from concourse.masks import make_identity
from concourse.tile import TileContext

B, NRX, NR, U, A, S, F = 4, 1, 32, 4, 8, 14, 1024
NCORES = 8
FS = F // NCORES        # 128 subcarriers per core
NB = 2                  # batch entries per chunk
NCH = B // NB           # chunks per core
M = U * NB * S          # 112 RE columns per chunk (u, b, s)
NP = 9                  # augmented planes: 8 matrix columns + rhs
F32 = mybir.dt.float32
AL = mybir.AluOpType

LAST_RESULTS = None     # BassKernelResults of the most recent run (for test.py)


def _off(j, i):
    """Free-dim offset of (plane j, row i) inside an H supertile."""
    return (j * A + i) * M


def _build():
    nc = bacc.Bacc(trn_type="TRN2")

    # Host-prepped layouts, chosen so every per-(chunk, i) DMA slice is
    # stride-collapsible: hd[i, u, b, s, j, f], yd[i, u, b, s, f],
    # out[i, u, b, s, f, c].  (i = matrix row, j = matrix column.)
    hdre = nc.dram_tensor("hd_re", [A, U, B, S, A, FS], F32, kind="ExternalInput")
    hdim = nc.dram_tensor("hd_im", [A, U, B, S, A, FS], F32, kind="ExternalInput")
    ydre = nc.dram_tensor("yd_re", [A, U, B, S, FS], F32, kind="ExternalInput")
    ydim = nc.dram_tensor("yd_im", [A, U, B, S, FS], F32, kind="ExternalInput")
    out = nc.dram_tensor("out", [A, U, B, S, FS, 2], F32, kind="ExternalOutput")

    with TileContext(nc) as tc:
        with (
            tc.tile_pool(name="consts", bufs=1) as consts,
            tc.tile_pool(name="supers", bufs=2) as supers,
            tc.tile_pool(name="work", bufs=1) as work,
            tc.tile_pool(name="stg", bufs=3) as stg,
            tc.tile_pool(name="stgo", bufs=3) as stgo,
            tc.tile_pool(name="psin", bufs=3, space="PSUM") as psin,
            tc.tile_pool(name="psy", bufs=2, space="PSUM") as psy_pool,
            tc.tile_pool(name="pso", bufs=2, space="PSUM") as pso_pool,
        ):
            ident = consts.tile([128, 128], F32)
            make_identity(nc, ident)

            for ci in range(NCH):
                b0 = ci * NB
                HRe = supers.tile([128, NP * A * M], F32, tag="HRe")
                HIm = supers.tile([128, NP * A * M], F32, tag="HIm")
                hsup = (HRe, HIm)

                def row(T, j, i):
                    return T[:, _off(j, i) : _off(j, i) + M]

                def rows3(T, j, i0, n):
                    base = _off(j, i0)
                    return T[:, base : base + n * M].rearrange(
                        "p (r c) -> p r c", r=n
                    )

                def bc(ap, n):
                    return ap[:, None, :].broadcast_to([128, n, M])

                # ---------------- load h ----------------
                for comp in range(2):
                    hsrc = (hdre, hdim)[comp]
                    for i in range(A):
                        stage = stg.tile([M, A * FS], F32, tag="stage")
                        src = hsrc[i, :, b0 : b0 + NB]
                        nc.sync.dma_start(stage, src)
                        for jg in range(2):
                            ps = psin.tile([128, 4 * M], F32, tag="psin")
                            for q in range(4):
                                j = jg * 4 + q
                                nc.tensor.transpose(
                                    ps[:, q * M : (q + 1) * M],
                                    stage[:, j * FS : (j + 1) * FS],
                                    ident[:M, :M],
                                )
                            base = _off(jg * 4, i)
                            dst = hsup[comp][:, base : base + 4 * A * M].rearrange(
                                "p (q c) -> p q c", q=4
                            )[:, :, :M]
                            src3 = ps.rearrange("p (q c) -> p q c", q=4)
                            nc.scalar.copy(dst, src3)

                # ---------------- load y ----------------
                for comp in range(2):
                    ysrc = (ydre, ydim)[comp]
                    for i in range(A):
                        sy = stg.tile([M, FS], F32, tag="stagey")
                        nc.sync.dma_start(sy, ysrc[i, :, b0 : b0 + NB])
                        py = psy_pool.tile([128, M], F32, tag="psy")
                        nc.tensor.transpose(py, sy, ident[:M, :M])
                        nc.scalar.copy(row(hsup[comp], 8, i), py)

                # ---------------- solve ----------------
                # INV holds the pivot reciprocals: ir block [0:A*M], ii block
                # [A*M:2*A*M], plus A*M padding so the (ir_k, ii_k) stride-
                # A*M pair view can be built by slice+rearrange for every k.
                INV = work.tile([128, 3 * A * M], F32, tag="INV")
                FRe = work.tile([128, (A - 1) * M], F32, tag="FRe")
                FIm = work.tile([128, (A - 1) * M], F32, tag="FIm")
                PAs = work.tile([128, 2 * (A - 1) * M], F32, tag="PAs")
                PBs = work.tile([128, 2 * (A - 1) * M], F32, tag="PBs")
                PCs = work.tile([128, 2 * (A - 1) * M], F32, tag="PCs")
                PDs = work.tile([128, 2 * (A - 1) * M], F32, tag="PDs")
                TD = work.tile([128, M], F32, tag="TD")
                TU = work.tile([128, M], F32, tag="TU")
                TR = work.tile([128, M], F32, tag="TR")

                def sc3(T, n):
                    return T[:, : n * M].rearrange("p (r c) -> p r c", r=n)

                def sc4(T, n):
                    # [128, 2, n, M] j-major view of scratch
                    return T[:, : 2 * n * M].rearrange(
                        "p (j r c) -> p j r c", j=2, r=n
                    )

                def sc_half(T, h, n):
                    return T[:, h * n * M : (h + 1) * n * M]

                def inv_pair(k, n=None):
                    # (ir_k, ii_k) as [128, 2, M]; broadcast over n rows if set
                    v = INV[:, k * M : k * M + 2 * A * M].rearrange(
                        "p (j c) -> p j c", j=2
                    )[:, :, :M]
                    if n is None:
                        return v
                    return v[:, :, None, :].broadcast_to([128, 2, n, M])

                def pair_rows(T, j0, i0, n):
                    # rows i0..i0+n of planes j0, j0+1 as [128, 2, n*M]
                    base = _off(j0, i0)
                    return T[:, base : base + 2 * A * M].rearrange(
                        "p (j c) -> p j c", j=2
                    )[:, :, : n * M]

                def pair_row_b(T, j0, k, n):
                    # row k of planes j0, j0+1, broadcast over n: [128,2,n,M]
                    base = _off(j0, k)
                    v = T[:, base : base + 2 * A * M].rearrange(
                        "p (j c) -> p j c", j=2
                    )[:, :, :M]
                    return v[:, :, None, :].broadcast_to([128, 2, n, M])

                def f_bcast(Ft, n):
                    v = Ft[:, : n * M].rearrange("p (r c) -> p r c", r=n)
                    return v[:, None, :, :].broadcast_to([128, 2, n, M])

                # forward elimination
                for k in range(A):
                    a = row(HRe, k, k)
                    b_ = row(HIm, k, k)
                    nc.scalar.square(TD, a)
                    nc.scalar.square(TU, b_)
                    nc.vector.tensor_add(TD, TD, TU)
                    nc.vector.reciprocal(TR, TD)
                    irk = INV[:, k * M : (k + 1) * M]
                    iik = INV[:, (A + k) * M : (A + k + 1) * M]
                    nc.vector.tensor_mul(irk, a, TR)
                    nc.vector.tensor_mul(iik, b_, TR)
                    n = A - 1 - k
                    if n == 0:
                        continue
                    # factors F = -H[i,k] * inv(p), via paired products:
                    #   P1 = (a*ir || a*ii),  P2 = (b*ir || b*ii)
                    car = rows3(HRe, k, k + 1, n)
                    cai = rows3(HIm, k, k + 1, n)
                    car4 = car[:, None, :, :].broadcast_to([128, 2, n, M])
                    cai4 = cai[:, None, :, :].broadcast_to([128, 2, n, M])
                    nc.vector.tensor_mul(sc4(PAs, n), car4, inv_pair(k, n))
                    nc.vector.tensor_mul(sc4(PBs, n), cai4, inv_pair(k, n))
                    fre = FRe[:, : n * M]
                    fim = FIm[:, : n * M]
                    # fre = -(a*ir + b*ii), fim = a*ii - b*ir
                    nc.vector.scalar_tensor_tensor(
                        fre, sc_half(PAs, 0, n), -1.0, sc_half(PBs, 1, n),
                        AL.mult, AL.subtract,
                    )
                    nc.vector.tensor_sub(
                        fim, sc_half(PAs, 1, n), sc_half(PBs, 0, n)
                    )
                    # eliminate column k from planes k+1..7 (paired) and y
                    js = list(range(k + 1, A))
                    groups = []
                    while len(js) >= 2:
                        groups.append((js[0], 2))
                        js = js[2:]
                    if js:
                        groups.append((js[0], 1))
                    groups.append((A, 1))  # y plane, always solo
                    for j0, w in groups:
                        if w == 2:
                            hr = pair_rows(HRe, j0, k + 1, n)
                            hi = pair_rows(HIm, j0, k + 1, n)
                            Br = pair_row_b(HRe, j0, k, n)
                            Bi = pair_row_b(HIm, j0, k, n)
                            fre_v = f_bcast(FRe, n)
                            fim_v = f_bcast(FIm, n)
                            pa, pb, pc, pd = (
                                sc4(t, n) for t in (PAs, PBs, PCs, PDs)
                            )
                            pa3, pb3, pc3, pd3 = (
                                t[:, : 2 * n * M].rearrange(
                                    "p (j c) -> p j c", j=2
                                )
                                for t in (PAs, PBs, PCs, PDs)
                            )
                        else:
                            hr = rows3(HRe, j0, k + 1, n)
                            hi = rows3(HIm, j0, k + 1, n)
                            Br = bc(row(HRe, j0, k), n)
                            Bi = bc(row(HIm, j0, k), n)
                            fre_v = sc3(FRe, n)
                            fim_v = sc3(FIm, n)
                            pa, pb, pc, pd = (
                                sc3(t, n) for t in (PAs, PBs, PCs, PDs)
                            )
                            pa3, pb3, pc3, pd3 = pa, pb, pc, pd
                        nc.vector.tensor_mul(pa, fre_v, Br)
                        nc.vector.tensor_mul(pb, fim_v, Bi)
                        nc.vector.tensor_mul(pc, fre_v, Bi)
                        nc.vector.tensor_mul(pd, fim_v, Br)
                        # H[i,j] += F*B (complex)
                        nc.vector.tensor_add(hr, hr, pa3)
                        nc.vector.tensor_sub(hr, hr, pb3)
                        nc.vector.tensor_add(hi, hi, pc3)
                        nc.vector.tensor_add(hi, hi, pd3)

                # back substitution (Jordan): x_k = y_k*invp, then clear col k
                for k in range(A - 1, -1, -1):
                    yr = row(HRe, 8, k)
                    yi = row(HIm, 8, k)
                    # P1 = (yr*ir || yr*ii), P2 = (yi*ir || yi*ii)
                    p1 = PAs[:, : 2 * M].rearrange("p (j c) -> p j c", j=2)
                    p2 = PBs[:, : 2 * M].rearrange("p (j c) -> p j c", j=2)
                    yr2 = yr[:, None, :].broadcast_to([128, 2, M])
                    yi2 = yi[:, None, :].broadcast_to([128, 2, M])
                    nc.vector.tensor_mul(p1, yr2, inv_pair(k))
                    nc.vector.tensor_mul(p2, yi2, inv_pair(k))
                    # x = y * conj(p)/|p|^2: xr = yr*ir + yi*ii, xi = yi*ir - yr*ii
                    nc.vector.tensor_add(yr, PAs[:, :M], PBs[:, M : 2 * M])
                    nc.vector.tensor_sub(yi, PBs[:, :M], PAs[:, M : 2 * M])
                    if k == 0:
                        continue
                    cr = rows3(HRe, k, 0, k)
                    ci_ = rows3(HIm, k, 0, k)
                    xrB = bc(yr, k)
                    xiB = bc(yi, k)
                    qa, qb, qc, qd = (sc3(t, k) for t in (PAs, PBs, PCs, PDs))
                    nc.vector.tensor_mul(qa, cr, xrB)
                    nc.vector.tensor_mul(qb, ci_, xiB)
                    nc.vector.tensor_mul(qc, cr, xiB)
                    nc.vector.tensor_mul(qd, ci_, xrB)
                    ytr = rows3(HRe, 8, 0, k)
                    yti = rows3(HIm, 8, 0, k)
                    # y_i -= H[i,k] * x_k
                    nc.vector.tensor_sub(ytr, ytr, qa)
                    nc.vector.tensor_add(ytr, ytr, qb)
                    nc.vector.tensor_sub(yti, yti, qc)
                    nc.vector.tensor_sub(yti, yti, qd)

                # ---------------- store ----------------
                for i in range(A):
                    so = stgo.tile([M, 2 * FS], F32, tag="so")
                    so3 = so.rearrange("p (f c) -> p f c", c=2)
                    for comp in range(2):
                        po = pso_pool.tile([M, FS], F32, tag="pso")
                        nc.tensor.transpose(
                            po, row(hsup[comp], 8, i), ident[:128, :128]
                        )
                        nc.scalar.copy(so3[:, :, comp], po)
                    dst = out[i, :, b0 : b0 + NB]
                    nc.sync.dma_start(dst, so)

    nc.finalize()
    return nc


_NC_CACHE = None


def _get_nc():
    global _NC_CACHE
    if _NC_CACHE is None:
        _NC_CACHE = _build()
    return _NC_CACHE


def _prep_core(y_re, y_im, h_re, h_im, c):
    """Host-side shard prep for core c: f-slice + block-diagonal extraction."""
    fsl = slice(c * FS, (c + 1) * FS)
    ue = np.arange(U)
    maps = {}
    for name, h in (("hd_re", h_re), ("hd_im", h_im)):
        h6 = h[:, 0, :, :, :, :, fsl].reshape(B, U, A, U, A, S, FS)
        hd = h6[:, ue, :, ue]              # [u, b, i, j, s, f]
        maps[name] = np.ascontiguousarray(
            hd.transpose(2, 0, 1, 4, 3, 5), dtype=np.float32
        )                                   # [i, u, b, s, j, f]
    for name, y in (("yd_re", y_re), ("yd_im", y_im)):
        y5 = y[:, 0, :, :, fsl].reshape(B, U, A, S, FS)   # [b, u, i, s, f]
        maps[name] = np.ascontiguousarray(
            y5.transpose(2, 1, 0, 3, 4), dtype=np.float32
        )                                   # [i, u, b, s, f]
    return maps


def kernel(y_re, y_im, h_re, h_im, **_ignored):
    global LAST_RESULTS
    y_re = np.asarray(y_re, dtype=np.float32)
    y_im = np.asarray(y_im, dtype=np.float32)
    h_re = np.asarray(h_re, dtype=np.float32)
    h_im = np.asarray(h_im, dtype=np.float32)

    nc = _get_nc()
    in_maps = [_prep_core(y_re, y_im, h_re, h_im, c) for c in range(NCORES)]
    trace = bool(int(os.environ.get("BD_TRACE", "0")))
    res = run_bass_kernel_spmd(
        nc, in_maps, core_ids=list(range(NCORES)), trace=trace
    )
    LAST_RESULTS = res
    outs = []
    for r in res.results:
        o = r["out"]                              # [i, u, b, s, f, c]
        o = o.transpose(2, 1, 0, 3, 4, 5)         # [b, u, i, s, f, c]
        outs.append(o.reshape(B, NR, S, FS, 2))
    full = np.concatenate(outs, axis=3)           # [B, NR, S, F, 2]
    return np.ascontiguousarray(full[:, None])    # [B, 1, NR, S, F, 2]


# revision 9
# speedup vs baseline: 5770.6714x; 5537.0841x over previous
"""Block-diagonal ZF equalizer (nn_BDEqualizer) as a Trainium2 Bass kernel.

Math: for every resource element (b, s, f) and UE u, solve the 8x8 complex
system H_u x_u = y_u where H_u[i, j] = h[b, 0, 8u+i, u, j, s, f] and
y_u[i] = y[b, 0, 8u+i, s, f].  Output x as [B, 1, 32, S, F, 2] (re/im last).

Strategy (data-parallel over the fft axis, per the sharding hint):
  - 8 cores, each owns a contiguous 128-subcarrier slice of F=1024.
  - Host pre-extracts the block-diagonal channel blocks (pure indexing) and
    ships per-core shards hd[B, U, 8, 8, S, 128] / yd[B, U, 8, S, 128].
  - On-chip layout: subcarriers on the 128 SBUF partitions, the other RE
    axes (u, b-pair, s) = 112 along the free dim.  Each of the 9 augmented
    matrix columns (8 of H + rhs) is a "plane" of 8 rows; every Gaussian
    elimination step is a full-width [128, n*112] elementwise op on the
    Vector engine, with per-RE pivot reciprocals.  Unpivoted LU + Jordan
    back-substitution, complex arithmetic as separate re/im tiles.
  - TensorE transposes move between the DMA-friendly [(u,b,s), f] staging
    layout and the compute layout [f, (u,b,s)]; ScalarE drains PSUM.
  - Two chunks (b in {0,1} then {2,3}) double-buffer load against compute.
"""

import os

import numpy as np

import concourse.bacc as bacc
import concourse.bass as bass
import concourse.mybir as mybir
from concourse.bass_utils import run_bass_kernel_spmd
from concourse.masks import make_identity
from concourse.tile import TileContext

B, NRX, NR, U, A, S, F = 4, 1, 32, 4, 8, 14, 1024
NCORES = 8
FS = F // NCORES        # 128 subcarriers per core
NB = 2                  # batch entries per chunk
NCH = B // NB           # chunks per core
M = U * NB * S          # 112 RE columns per chunk (u, b, s)
NP = 9                  # augmented planes: 8 matrix columns + rhs
F32 = mybir.dt.float32
AL = mybir.AluOpType

LAST_RESULTS = None     # BassKernelResults of the most recent run (for test.py)


def _off(j, i):
    """Free-dim offset of (plane j, row i) inside an H supertile."""
    return (j * A + i) * M


def _build():
    nc = bacc.Bacc(trn_type="TRN2")

    # Host-prepped layouts, chosen so every per-(chunk, i) DMA slice is
    # stride-collapsible: hd[i, u, b, s, j, f], yd[i, u, b, s, f],
    # out[i, u, b, s, f, c].  (i = matrix row, j = matrix column.)
    hdre = nc.dram_tensor("hd_re", [A, U, B, S, A, FS], F32, kind="ExternalInput")
    hdim = nc.dram_tensor("hd_im", [A, U, B, S, A, FS], F32, kind="ExternalInput")
    ydre = nc.dram_tensor("yd_re", [A, U, B, S, FS], F32, kind="ExternalInput")
    ydim = nc.dram_tensor("yd_im", [A, U, B, S, FS], F32, kind="ExternalInput")
    out = nc.dram_tensor("out", [A, U, B, S, FS, 2], F32, kind="ExternalOutput")

    with TileContext(nc) as tc:
        with (
            tc.tile_pool(name="consts", bufs=1) as consts,
            tc.tile_pool(name="supers", bufs=2) as supers,
            tc.tile_pool(name="work", bufs=1) as work,
            tc.tile_pool(name="stg", bufs=3) as stg,
            tc.tile_pool(name="stgo", bufs=3) as stgo,
            tc.tile_pool(name="psin", bufs=3, space="PSUM") as psin,
            tc.tile_pool(name="psy", bufs=2, space="PSUM") as psy_pool,
            tc.tile_pool(name="pso", bufs=2, space="PSUM") as pso_pool,
        ):
            ident = consts.tile([128, 128], F32)
            make_identity(nc, ident)

            for ci in range(NCH):
                b0 = ci * NB
                HRe = supers.tile([128, NP * A * M], F32, tag="HRe")
                HIm = supers.tile([128, NP * A * M], F32, tag="HIm")
                hsup = (HRe, HIm)

                def row(T, j, i):
                    return T[:, _off(j, i) : _off(j, i) + M]

                def rows3(T, j, i0, n):
                    base = _off(j, i0)
                    return T[:, base : base + n * M].rearrange(
                        "p (r c) -> p r c", r=n
                    )

                def bc(ap, n):
                    return ap[:, None, :].broadcast_to([128, n, M])

                # ---------------- load h ----------------
                for comp in range(2):
                    hsrc = (hdre, hdim)[comp]
                    for i in range(A):
                        stage = stg.tile([M, A * FS], F32, tag="stage")
                        src = hsrc[i, :, b0 : b0 + NB]
                        nc.sync.dma_start(stage, src)
                        for jg in range(2):
                            ps = psin.tile([128, 4 * M], F32, tag="psin")
                            for q in range(4):
                                j = jg * 4 + q
                                nc.tensor.transpose(
                                    ps[:, q * M : (q + 1) * M],
                                    stage[:, j * FS : (j + 1) * FS],
                                    ident[:M, :M],
                                )
                            base = _off(jg * 4, i)
                            dst = hsup[comp][:, base : base + 4 * A * M].rearrange(
                                "p (q c) -> p q c", q=4
                            )[:, :, :M]
                            src3 = ps.rearrange("p (q c) -> p q c", q=4)
                            nc.scalar.copy(dst, src3)

                # ---------------- load y ----------------
                for comp in range(2):
                    ysrc = (ydre, ydim)[comp]
                    for i in range(A):
                        sy = stg.tile([M, FS], F32, tag="stagey")
                        nc.sync.dma_start(sy, ysrc[i, :, b0 : b0 + NB])
                        py = psy_pool.tile([128, M], F32, tag="psy")
                        nc.tensor.transpose(py, sy, ident[:M, :M])
                        nc.scalar.copy(row(hsup[comp], 8, i), py)

                # ---------------- solve ----------------
                # INV holds the pivot reciprocals: ir block [0:A*M], ii block
                # [A*M:2*A*M], plus A*M padding so the (ir_k, ii_k) stride-
                # A*M pair view can be built by slice+rearrange for every k.
                INV = work.tile([128, 3 * A * M], F32, tag="INV")
                FRe = work.tile([128, (A - 1) * M], F32, tag="FRe")
                FIm = work.tile([128, (A - 1) * M], F32, tag="FIm")
                PAs = work.tile([128, 2 * (A - 1) * M], F32, tag="PAs")
                PBs = work.tile([128, 2 * (A - 1) * M], F32, tag="PBs")
                PCs = work.tile([128, 2 * (A - 1) * M], F32, tag="PCs")
                PDs = work.tile([128, 2 * (A - 1) * M], F32, tag="PDs")
                TD = work.tile([128, M], F32, tag="TD")
                TU = work.tile([128, M], F32, tag="TU")
                TR = work.tile([128, M], F32, tag="TR")

                def sc3(T, n):
                    return T[:, : n * M].rearrange("p (r c) -> p r c", r=n)

                def sc4(T, n):
                    # [128, 2, n, M] j-major view of scratch
                    return T[:, : 2 * n * M].rearrange(
                        "p (j r c) -> p j r c", j=2, r=n
                    )

                def sc_half(T, h, n):
                    return T[:, h * n * M : (h + 1) * n * M]

                def inv_pair(k, n=None):
                    # (ir_k, ii_k) as [128, 2, M]; broadcast over n rows if set
                    v = INV[:, k * M : k * M + 2 * A * M].rearrange(
                        "p (j c) -> p j c", j=2
                    )[:, :, :M]
                    if n is None:
                        return v
                    return v[:, :, None, :].broadcast_to([128, 2, n, M])

                def pair_rows(T, j0, i0, n):
                    # rows i0..i0+n of planes j0, j0+1 as [128, 2, n*M]
                    base = _off(j0, i0)
                    return T[:, base : base + 2 * A * M].rearrange(
                        "p (j c) -> p j c", j=2
                    )[:, :, : n * M]

                def pair_row_b(T, j0, k, n):
                    # row k of planes j0, j0+1, broadcast over n: [128,2,n,M]
                    base = _off(j0, k)
                    v = T[:, base : base + 2 * A * M].rearrange(
                        "p (j c) -> p j c", j=2
                    )[:, :, :M]
                    return v[:, :, None, :].broadcast_to([128, 2, n, M])

                def f_bcast(Ft, n):
                    v = Ft[:, : n * M].rearrange("p (r c) -> p r c", r=n)
                    return v[:, None, :, :].broadcast_to([128, 2, n, M])

                # forward elimination
                for k in range(A):
                    a = row(HRe, k, k)
                    b_ = row(HIm, k, k)
                    nc.scalar.square(TD, a)
                    nc.scalar.square(TU, b_)
                    nc.vector.tensor_add(TD, TD, TU)
                    # ~2 ULP, ~2.8x faster than the exact iterative divide;
                    # |p|^2 is strictly positive (denorm only if |p| < 1e-19).
                    nc.vector.reciprocal_approx_accurate(TR, TD, scratch=TU)
                    irk = INV[:, k * M : (k + 1) * M]
                    iik = INV[:, (A + k) * M : (A + k + 1) * M]
                    nc.vector.tensor_mul(irk, a, TR)
                    nc.vector.tensor_mul(iik, b_, TR)
                    n = A - 1 - k
                    if n == 0:
                        continue
                    # factors F = -H[i,k] * inv(p), via paired products:
                    #   P1 = (a*ir || a*ii),  P2 = (b*ir || b*ii)
                    car = rows3(HRe, k, k + 1, n)
                    cai = rows3(HIm, k, k + 1, n)
                    car4 = car[:, None, :, :].broadcast_to([128, 2, n, M])
                    cai4 = cai[:, None, :, :].broadcast_to([128, 2, n, M])
                    nc.vector.tensor_mul(sc4(PAs, n), car4, inv_pair(k, n))
                    nc.vector.tensor_mul(sc4(PBs, n), cai4, inv_pair(k, n))
                    fre = FRe[:, : n * M]
                    fim = FIm[:, : n * M]
                    # fre = -(a*ir + b*ii), fim = a*ii - b*ir
                    nc.vector.scalar_tensor_tensor(
                        fre, sc_half(PAs, 0, n), -1.0, sc_half(PBs, 1, n),
                        AL.mult, AL.subtract,
                    )
                    nc.vector.tensor_sub(
                        fim, sc_half(PAs, 1, n), sc_half(PBs, 0, n)
                    )
                    # eliminate column k from planes k+1..7 (paired) and y
                    js = list(range(k + 1, A))
                    groups = []
                    while len(js) >= 2:
                        groups.append((js[0], 2))
                        js = js[2:]
                    if js:
                        groups.append((js[0], 1))
                    groups.append((A, 1))  # y plane, always solo
                    for j0, w in groups:
                        if w == 2:
                            hr = pair_rows(HRe, j0, k + 1, n)
                            hi = pair_rows(HIm, j0, k + 1, n)
                            Br = pair_row_b(HRe, j0, k, n)
                            Bi = pair_row_b(HIm, j0, k, n)
                            fre_v = f_bcast(FRe, n)
                            fim_v = f_bcast(FIm, n)
                            pa, pb, pc, pd = (
                                sc4(t, n) for t in (PAs, PBs, PCs, PDs)
                            )
                            pa3, pb3, pc3, pd3 = (
                                t[:, : 2 * n * M].rearrange(
                                    "p (j c) -> p j c", j=2
                                )
                                for t in (PAs, PBs, PCs, PDs)
                            )
                        else:
                            hr = rows3(HRe, j0, k + 1, n)
                            hi = rows3(HIm, j0, k + 1, n)
                            Br = bc(row(HRe, j0, k), n)
                            Bi = bc(row(HIm, j0, k), n)
                            fre_v = sc3(FRe, n)
                            fim_v = sc3(FIm, n)
                            pa, pb, pc, pd = (
                                sc3(t, n) for t in (PAs, PBs, PCs, PDs)
                            )
                            pa3, pb3, pc3, pd3 = pa, pb, pc, pd
                        nc.vector.tensor_mul(pa, fre_v, Br)
                        nc.vector.tensor_mul(pb, fim_v, Bi)
                        nc.vector.tensor_mul(pc, fre_v, Bi)
                        nc.vector.tensor_mul(pd, fim_v, Br)
                        # H[i,j] += F*B (complex)
                        nc.vector.tensor_add(hr, hr, pa3)
                        nc.vector.tensor_sub(hr, hr, pb3)
                        nc.vector.tensor_add(hi, hi, pc3)
                        nc.vector.tensor_add(hi, hi, pd3)

                # back substitution (Jordan): x_k = y_k*invp, then clear col k
                for k in range(A - 1, -1, -1):
                    yr = row(HRe, 8, k)
                    yi = row(HIm, 8, k)
                    # P1 = (yr*ir || yr*ii), P2 = (yi*ir || yi*ii)
                    p1 = PAs[:, : 2 * M].rearrange("p (j c) -> p j c", j=2)
                    p2 = PBs[:, : 2 * M].rearrange("p (j c) -> p j c", j=2)
                    yr2 = yr[:, None, :].broadcast_to([128, 2, M])
                    yi2 = yi[:, None, :].broadcast_to([128, 2, M])
                    nc.vector.tensor_mul(p1, yr2, inv_pair(k))
                    nc.vector.tensor_mul(p2, yi2, inv_pair(k))
                    # x = y * conj(p)/|p|^2: xr = yr*ir + yi*ii, xi = yi*ir - yr*ii
                    nc.vector.tensor_add(yr, PAs[:, :M], PBs[:, M : 2 * M])
                    nc.vector.tensor_sub(yi, PBs[:, :M], PAs[:, M : 2 * M])
                    if k == 0:
                        continue
                    cr = rows3(HRe, k, 0, k)
                    ci_ = rows3(HIm, k, 0, k)
                    xrB = bc(yr, k)
                    xiB = bc(yi, k)
                    qa, qb, qc, qd = (sc3(t, k) for t in (PAs, PBs, PCs, PDs))
                    nc.vector.tensor_mul(qa, cr, xrB)
                    nc.vector.tensor_mul(qb, ci_, xiB)
                    nc.vector.tensor_mul(qc, cr, xiB)
                    nc.vector.tensor_mul(qd, ci_, xrB)
                    ytr = rows3(HRe, 8, 0, k)
                    yti = rows3(HIm, 8, 0, k)
                    # y_i -= H[i,k] * x_k
                    nc.vector.tensor_sub(ytr, ytr, qa)
                    nc.vector.tensor_add(ytr, ytr, qb)
                    nc.vector.tensor_sub(yti, yti, qc)
                    nc.vector.tensor_sub(yti, yti, qd)

                # ---------------- store ----------------
                for i in range(A):
                    so = stgo.tile([M, 2 * FS], F32, tag="so")
                    so3 = so.rearrange("p (f c) -> p f c", c=2)
                    for comp in range(2):
                        po = pso_pool.tile([M, FS], F32, tag="pso")
                        nc.tensor.transpose(
                            po, row(hsup[comp], 8, i), ident[:128, :128]
                        )
                        nc.scalar.copy(so3[:, :, comp], po)
                    dst = out[i, :, b0 : b0 + NB]
                    nc.sync.dma_start(dst, so)

    nc.finalize()
    return nc


_NC_CACHE = None


def _get_nc():
    global _NC_CACHE
    if _NC_CACHE is None:
        _NC_CACHE = _build()
    return _NC_CACHE


def _prep_core(y_re, y_im, h_re, h_im, c):
    """Host-side shard prep for core c: f-slice + block-diagonal extraction."""
    fsl = slice(c * FS, (c + 1) * FS)
    ue = np.arange(U)
    maps = {}
    for name, h in (("hd_re", h_re), ("hd_im", h_im)):
        h6 = h[:, 0, :, :, :, :, fsl].reshape(B, U, A, U, A, S, FS)
        hd = h6[:, ue, :, ue]              # [u, b, i, j, s, f]
        maps[name] = np.ascontiguousarray(
            hd.transpose(2, 0, 1, 4, 3, 5), dtype=np.float32
        )                                   # [i, u, b, s, j, f]
    for name, y in (("yd_re", y_re), ("yd_im", y_im)):
        y5 = y[:, 0, :, :, fsl].reshape(B, U, A, S, FS)   # [b, u, i, s, f]
        maps[name] = np.ascontiguousarray(
            y5.transpose(2, 1, 0, 3, 4), dtype=np.float32
        )                                   # [i, u, b, s, f]
    return maps


def kernel(y_re, y_im, h_re, h_im, **_ignored):
    global LAST_RESULTS
    y_re = np.asarray(y_re, dtype=np.float32)
    y_im = np.asarray(y_im, dtype=np.float32)
    h_re = np.asarray(h_re, dtype=np.float32)
    h_im = np.asarray(h_im, dtype=np.float32)

    nc = _get_nc()
    in_maps = [_prep_core(y_re, y_im, h_re, h_im, c) for c in range(NCORES)]
    trace = bool(int(os.environ.get("BD_TRACE", "0")))
    res = run_bass_kernel_spmd(
        nc, in_maps, core_ids=list(range(NCORES)), trace=trace
    )
    LAST_RESULTS = res
    outs = []
    for r in res.results:
        o = r["out"]                              # [i, u, b, s, f, c]
        o = o.transpose(2, 1, 0, 3, 4, 5)         # [b, u, i, s, f, c]
        outs.append(o.reshape(B, NR, S, FS, 2))
    full = np.concatenate(outs, axis=3)           # [B, NR, S, F, 2]
    return np.ascontiguousarray(full[:, None])    # [B, 1, NR, S, F, 2]
